# revision 6
# baseline (speedup 1.0000x reference)
"""Trainium2 Bass kernel v2 for nn_Discriminator (GNN message passing).

Math: raw[e] = s[e0] + s[e1] + b_edge, s = relu(emb@W+b) @ w_sym,
      out = sigmoid(logit(eps) + raw).

v2 strategy (vs v1's 2 ap_gather indices per edge):
  - Shard edges by e0's owner range (6250 nodes/core). Sort within core by e0.
  - Each node's edges packed into fixed-size instance bins:
      region A: first min(deg,8) edges -> 8 slots on the node's own row
                (node-ordered; doubles as the AllGather table payload)
      region B: extra full-8 bins, region C: leftover pairs (2 slots)
  - The GEMM computes s per INSTANCE (embedding rows duplicated on host), so
    the e0 side becomes a pure affine broadcast of the GEMM output: no gather.
  - Only s[e1] needs a real gather: ONE ap_gather index per slot (half of v1).
  - AllGather ships only region A (node-ordered, 6656 f32/core) right after
    the first 13 GEMM slabs; B/C slabs + broadcasts overlap the gather.
"""

import os
import sys
import types
import contextlib
import ctypes

sys.path.insert(0, "/opt/trn_rl_repo")

import numpy as np

import concourse.bass as bass
import concourse.mybir as mybir
import concourse.tile as tile
import concourse.bacc as bacc
from concourse.bass_utils import run_bass_kernel_spmd

# ---------------------------------------------------------------- constants
N, IN_DIM, HID, E = 50000, 512, 256, 800000
NCORES = 8
BIAS = 0.0001

NLOC = N // NCORES            # 6250 nodes per core
AQI = 52                      # A instances per partition (128*52 = 6656)
NA = 128 * AQI                # 6656 A rows (13 slabs)
BQI = 30                      # B instances per partition (3840 cap)
NB = 128 * BQI
CQI = 104                     # C instances per partition (13312, 26 slabs)
NC_ = 128 * CQI
INST = NA + NB + NC_          # 23808 real GEMM rows
SLABS = (INST + 511) // 512   # 47 slabs (rows padded to 24064)
INSTP = SLABS * 512
ACOL = 8 * AQI                # 416
BCOL = 8 * BQI                # 224
CCOL = 2 * CQI                # 208
FW = ACOL + BCOL + CCOL       # 840 slot columns
SLOTS = 128 * FW              # 107520 slots per core
KG = 16 * FW                  # 13440 gather idx per gpsimd group
NPAD = NA * NCORES            # 53248 table entries
CHUNK = NPAD // 16            # 3328 per-partition chunk
GSEG = 3456                   # max idx per ap_gather call (4*FW, 32-aligned slices)
WIN = 512                     # select window

f32 = mybir.dt.float32
f32r = mybir.dt.float32r
bf16 = mybir.dt.bfloat16


def _install_ntff_hook():
    """Provide antenv.axon_hooks (absent in this image) so trace=True works."""
    if "antenv.axon_hooks" in sys.modules:
        return
    try:
        lib = ctypes.CDLL("/opt/axon/libaxon_pjrt.so")
    except OSError:
        return
    if not hasattr(lib, "axon_start_nrt_profile"):
        return
    lib.axon_start_nrt_profile.argtypes = [ctypes.POINTER(ctypes.c_int64), ctypes.c_size_t]
    lib.axon_start_nrt_profile.restype = ctypes.c_int64
    lib.axon_stop_nrt_profile.argtypes = [ctypes.c_char_p]
    lib.axon_stop_nrt_profile.restype = ctypes.c_int64

    @contextlib.contextmanager
    def _hook(output_dir, device_ids):
        import jax
        jax.devices()
        if device_ids:
            ids = (ctypes.c_int64 * len(device_ids))(*device_ids)
            rc = lib.axon_start_nrt_profile(ids, len(device_ids))
        else:
            rc = lib.axon_start_nrt_profile(None, 0)
        if rc != 0:
            raise RuntimeError(f"axon_start_nrt_profile rc={rc}")
        try:
            yield
        finally:
            n = lib.axon_stop_nrt_profile(str(output_dir).encode())
            print(f"profile: {n} file(s) written to {output_dir}", file=sys.stderr)

    mod = types.ModuleType("antenv.axon_hooks")
    mod.get_axon_ntff_profile_hook = lambda: _hook
    mod.set_axon_ntff_profile_hook = lambda h: None
    sys.modules["antenv.axon_hooks"] = mod


_install_ntff_hook()

_PROGRAM_CACHE = {}


def _build_program(debug: bool):
    nc = _build_body(debug)
    nc.finalize()
    return nc


def _build_body(debug: bool):
    nc = bacc.Bacc(None)

    embT = nc.dram_tensor("embT", [4 * SLABS * 128, 512], bf16, kind="ExternalInput")
    Wt = nc.dram_tensor("Wt", [IN_DIM, HID], bf16, kind="ExternalInput")
    bias2 = nc.dram_tensor("bias2", [128, 2], f32, kind="ExternalInput")
    wsym2 = nc.dram_tensor("wsym2", [128, 2], bf16, kind="ExternalInput")
    bedge = nc.dram_tensor("bedge", [128, 3], f32, kind="ExternalInput")
    iota16 = nc.dram_tensor("iota16", [128, 1], f32, kind="ExternalInput")
    e8 = nc.dram_tensor("e8", [128, 8], f32r, kind="ExternalInput")
    rw = nc.dram_tensor("rw", [128, KG // 16], mybir.dt.int16, kind="ExternalInput")
    cu = nc.dram_tensor("cu", [128, KG], mybir.dt.uint8, kind="ExternalInput")
    uu = nc.dram_tensor("uu", [128, FW], f32, kind="ExternalInput")
    out = nc.dram_tensor("out", [128, FW], f32, kind="ExternalOutput")
    if debug:
        sdbg = nc.dram_tensor("sdbg", [16, CHUNK], f32, kind="ExternalOutput")
        rawdbg = nc.dram_tensor("rawdbg", [128, FW], f32, kind="ExternalOutput")

    with tile.TileContext(nc) as tc:
        with (
            tc.tile_pool(name="const", bufs=1) as constp,
            tc.tile_pool(name="w", bufs=1) as wp,
            tc.tile_pool(name="emb", bufs=4) as embp,
            tc.tile_pool(name="h", bufs=2) as hp,
            tc.tile_pool(name="s", bufs=2) as sp,
            tc.tile_pool(name="tab", bufs=1) as tabp,
            tc.tile_pool(name="gat", bufs=2) as gatp,
            tc.tile_pool(name="win", bufs=3) as winp,
            tc.tile_pool(name="fin", bufs=1) as finp,
            tc.tile_pool(name="psA", bufs=2, space="PSUM") as psA,
            tc.tile_pool(name="psS", bufs=2, space="PSUM") as psS,
            tc.tile_pool(name="psR", bufs=2, space="PSUM") as psR,
            tc.tile_pool(name="dram", bufs=1, space="DRAM") as dramp,
        ):
            # ---------------- constants into SBUF
            t_bias2 = constp.tile([128, 2], f32)
            nc.sync.dma_start(t_bias2[:], bias2[:])
            t_wsym2 = constp.tile([128, 2], bf16)
            nc.sync.dma_start(t_wsym2[:], wsym2[:])
            t_bedge = constp.tile([128, 3], f32)
            nc.sync.dma_start(t_bedge[:], bedge[:])
            t_iota16 = constp.tile([128, 1], f32)
            nc.sync.dma_start(t_iota16[:], iota16[:])
            t_e8 = constp.tile([128, 8], f32r)
            nc.sync.dma_start(t_e8[:], e8[:])
            t_W = wp.tile([128, 4 * HID], bf16)
            for k in range(4):
                nc.sync.dma_start(t_W[:, k * HID:(k + 1) * HID], Wt[128 * k:128 * (k + 1), :])
            t_rw = constp.tile([128, KG // 16], mybir.dt.int16)
            nc.sync.dma_start(t_rw[:], rw[:])

            # logit(eps) terms depend only on u: compute up front
            t_u = finp.tile([128, FW], f32)
            nc.sync.dma_start(t_u[:], uu[:])
            a = 1.0 - 2.0 * BIAS
            t_l1 = finp.tile([128, FW], f32)
            nc.scalar.activation(t_l1[:], t_u[:], mybir.ActivationFunctionType.Ln,
                                 bias=t_bedge[:, 1:2], scale=-a)
            t_l2 = finp.tile([128, FW], f32)
            nc.scalar.activation(t_l2[:], t_u[:], mybir.ActivationFunctionType.Ln,
                                 bias=t_bedge[:, 2:3], scale=a)
            t_gate = finp.tile([128, FW], f32)
            nc.vector.tensor_sub(t_gate[:], t_l1[:], t_l2[:])

            # ---------------- stage A: per-instance s = relu(emb @ W + b) @ w_sym
            d_sin = dramp.tile([INSTP], f32)
            ASLABS = NA // 512

            def gemm_slab(si):
                t_embs = embp.tile([128, 4 * 512], bf16, tag="embs")
                nc.sync.dma_start(
                    t_embs[:].rearrange("p (k c) -> p k c", k=4),
                    embT[512 * si:512 * (si + 1), :].rearrange("(k p) c -> p k c", k=4),
                )
                ps_ss = []
                for H in range(2):
                    ps_h = psA.tile([128, 512], f32, tag="ps_h")
                    for k in range(4):
                        nc.tensor.matmul(
                            ps_h[:, :],
                            lhsT=t_W[:, k * HID + 128 * H:k * HID + 128 * (H + 1)],
                            rhs=t_embs[:, k * 512:(k + 1) * 512],
                            start=(k == 0),
                            stop=(k == 3),
                        )
                    t_h = hp.tile([128, 512], bf16, tag="h")
                    nc.scalar.activation(
                        t_h[:, :], ps_h[:, :],
                        mybir.ActivationFunctionType.Relu,
                        bias=t_bias2[:, H:H + 1],
                    )
                    ps_s = psS.tile([1, 512], f32, tag=f"ps_s{H}")
                    nc.tensor.matmul(
                        ps_s[:1, :],
                        lhsT=t_wsym2[:, H:H + 1],
                        rhs=t_h[:, :],
                        start=True,
                        stop=True,
                    )
                    ps_ss.append(ps_s)
                t_s0 = sp.tile([1, 512], f32, tag="s0")
                nc.scalar.copy(t_s0[:1, :], ps_ss[0][:1, :])
                t_sst = sp.tile([1, 512], f32, tag="sst")
                nc.vector.tensor_tensor(
                    out=t_sst[:1, :], in0=t_s0[:1, :], in1=ps_ss[1][:1, :],
                    op=mybir.AluOpType.add,
                )
                nc.sync.dma_start(
                    d_sin[512 * si:512 * (si + 1)].rearrange("(a b) -> a b", a=1),
                    t_sst[:1, :],
                )

            for si in range(ASLABS):
                gemm_slab(si)

            # ---------------- stage B: AllGather region A (first 13 slabs)
            d_sout = dramp.tile([16, CHUNK], f32)
            nc.gpsimd.collective_compute(
                "AllGather",
                mybir.AluOpType.bypass,
                ins=[d_sin[0:NA].opt()],
                outs=[d_sout[:].opt()],
                replica_groups=[list(range(NCORES))],
            )
            t_tab = tabp.tile([128, CHUNK], f32)
            for g in range(8):
                nc.sync.dma_start(t_tab[16 * g:16 * (g + 1), :], d_sout[:, :])
            if debug:
                nc.sync.dma_start(sdbg[:, :], d_sout[:, :])

            for si in range(ASLABS, SLABS):
                gemm_slab(si)

            # ---------------- stage C: ap_gather of s[e1] (1 idx per slot)
            d_rawsp = dramp.tile([8, KG], f32)
            t_raw = finp.tile([128, FW], f32)
            tab3 = t_tab[:].rearrange("p (n d) -> p n d", d=1)
            SEGSCHED = [3456, 3456, 3456, 1728, 864, 864]
            assert sum(SEGSCHED) == KG
            s0 = 0
            for sl in SEGSCHED:
                t_g = gatp.tile([128, GSEG], f32, tag="g1")
                nc.gpsimd.ap_gather(
                    t_g[:, :sl].rearrange("p (n d) -> p n d", d=1),
                    tab3,
                    t_rw[:, s0 // 16:(s0 + sl) // 16],
                    channels=128,
                    num_elems=CHUNK,
                    d=1,
                    num_idxs=sl,
                )
                t_cu = gatp.tile([128, GSEG], mybir.dt.uint8, tag="cu")
                nc.scalar.dma_start(t_cu[:, :sl], cu[:, s0:s0 + sl])
                for lo in range(0, sl, WIN):
                    w = min(WIN, sl - lo)
                    t_cf = winp.tile([128, WIN], f32, tag="cf")
                    nc.vector.tensor_copy(t_cf[:, :w], t_cu[:, lo:lo + w])
                    t_m = winp.tile([128, WIN], f32r, tag="m")
                    nc.vector.scalar_tensor_tensor(
                        t_m[:, :w],
                        in0=t_cf[:, :w],
                        scalar=t_iota16[:, 0:1],
                        in1=t_g[:, lo:lo + w],
                        op0=mybir.AluOpType.is_equal,
                        op1=mybir.AluOpType.mult,
                    )
                    ps_r = psR.tile([8, WIN], f32, tag="ps_r")
                    nc.tensor.matmul(
                        ps_r[:, :w],
                        lhsT=t_e8[:],
                        rhs=t_m[:, :w],
                        start=True,
                        stop=True,
                    )
                    t_rwv = winp.tile([8, WIN], f32, tag="rwv")
                    nc.vector.tensor_copy(t_rwv[:, :w], ps_r[:, :w])
                    nc.scalar.dma_start(d_rawsp[:, s0 + lo:s0 + lo + w], t_rwv[:, :w])
                # fold this segment (aligned to FW rows): stream rows c0:c1
                c0, c1 = s0 // FW, (s0 + sl) // FW
                for g in range(8):
                    nc.scalar.dma_start(
                        t_raw[16 * g + c0:16 * g + c1, :],
                        d_rawsp[g, s0:s0 + sl].rearrange("(c f) -> c f", f=FW),
                    )
                s0 += sl

            # ---------------- stage D: e0 side via affine broadcast
            t_slA = finp.tile([128, AQI], f32)
            nc.sync.dma_start(t_slA[:], d_sin[0:NA].rearrange("(p q) -> p q", p=128))
            t_slB = finp.tile([128, BQI], f32)
            nc.sync.dma_start(t_slB[:], d_sin[NA:NA + NB].rearrange("(p q) -> p q", p=128))
            t_slC = finp.tile([128, CQI], f32)
            nc.sync.dma_start(t_slC[:], d_sin[NA + NB:INST].rearrange("(p q) -> p q", p=128))
            t_e0 = finp.tile([128, FW], f32)
            for t in range(8):
                nc.vector.tensor_copy(t_e0[:, t:ACOL:8], t_slA[:])
                nc.vector.tensor_copy(t_e0[:, ACOL + t:ACOL + BCOL:8], t_slB[:])
            for t in range(2):
                nc.vector.tensor_copy(t_e0[:, ACOL + BCOL + t:FW:2], t_slC[:])

            t_ge = finp.tile([128, FW], f32)
            nc.vector.tensor_add(t_ge[:], t_gate[:], t_e0[:])
            if debug:
                t_sum = finp.tile([128, FW], f32)
                nc.vector.tensor_add(t_sum[:], t_raw[:], t_e0[:])
                nc.sync.dma_start(rawdbg[:, :], t_sum[:, :])

            # ---------------- stage E: logit(eps) + raw, sigmoid
            t_gate2 = finp.tile([128, FW], f32)
            nc.vector.tensor_add(t_gate2[:], t_ge[:], t_raw[:])
            t_out = finp.tile([128, FW], f32)
            nc.scalar.activation(t_out[:], t_gate2[:], mybir.ActivationFunctionType.Sigmoid,
                                 bias=t_bedge[:, 0:1])
            nc.sync.dma_start(out[:, :], t_out[:])

    return nc


def _prep_inputs(embedding, edges, u, W_emb, b_emb, W_edge, b_edge):
    """Host-side sharding / layout prep. Returns per-core input maps + slot map."""
    embedding = np.ascontiguousarray(np.asarray(embedding, dtype=np.float32))
    edges = np.asarray(edges).astype(np.int64)
    u = np.asarray(u, dtype=np.float32)
    W_emb = np.asarray(W_emb, dtype=np.float32)
    b_emb = np.asarray(b_emb, dtype=np.float32)
    W_edge = np.asarray(W_edge, dtype=np.float32)
    b_edge = np.asarray(b_edge, dtype=np.float32)

    wsym = 0.5 * (W_edge[:HID, 0] + W_edge[HID:, 0])
    bias2 = b_emb.reshape(2, 128).T.copy()           # [128, 2]
    wsym2 = wsym.reshape(2, 128).T.astype(np.float32)
    import ml_dtypes
    wsym2_bf = wsym2.astype(ml_dtypes.bfloat16)
    bedge = np.tile(np.array([[b_edge[0], 1.0 - BIAS, BIAS]], np.float32), (128, 1))
    iota16 = (np.arange(128) % 16).astype(np.float32)[:, None]
    e8 = (np.arange(128)[:, None] // 16 == np.arange(8)[None, :]).astype(np.float32)
    W_bf = W_emb.astype(ml_dtypes.bfloat16)

    owner = edges[0] // NLOC
    in_maps = []
    slot_maps = []
    for c in range(NCORES):
        gids = np.nonzero(owner == c)[0]
        e0loc = (edges[0, gids] - NLOC * c).astype(np.int64)
        order = np.argsort(e0loc, kind="stable")
        gids = gids[order]
        e0loc = e0loc[order]
        e1g = edges[1, gids]
        m = len(gids)
        deg = np.bincount(e0loc, minlength=NLOC)
        start = np.concatenate([[0], np.cumsum(deg)[:-1]])

        # per-edge rank within its node
        rank = np.arange(m) - start[e0loc]

        # slot assignment: (p, col) per edge
        p_arr = np.empty(m, np.int64)
        col_arr = np.empty(m, np.int64)

        # region A: rank < 8
        selA = rank < 8
        nA_ = e0loc[selA]
        p_arr[selA] = nA_ // AQI
        col_arr[selA] = 8 * (nA_ % AQI) + rank[selA]

        # region B: 8 <= rank < 8 + 8*b_n
        r1 = np.maximum(deg - 8, 0)
        bcount = r1 // 8
        cumB = np.concatenate([[0], np.cumsum(bcount)[:-1]])
        nBc = int(bcount.sum())
        selB = (rank >= 8) & (rank < 8 + 8 * bcount[e0loc])
        rB = rank[selB] - 8
        iB = cumB[e0loc[selB]] + rB // 8
        p_arr[selB] = iB // BQI
        col_arr[selB] = ACOL + 8 * (iB % BQI) + rB % 8

        # region C: the rest, pairs
        ccount = np.ceil((r1 % 8) / 2).astype(np.int64)
        cumC = np.concatenate([[0], np.cumsum(ccount)[:-1]])
        nCc = int(ccount.sum())
        selC = ~(selA | selB)
        rC = rank[selC] - 8 - 8 * bcount[e0loc[selC]]
        iC = cumC[e0loc[selC]] + rC // 2
        p_arr[selC] = iC // CQI
        col_arr[selC] = ACOL + BCOL + 2 * (iC % CQI) + rC % 2

        assert nBc <= NB, (c, nBc)
        assert nCc <= NC_, (c, nCc)

        # per-slot tensors
        slot_gid = -np.ones((128, FW), np.int64)
        slot_gid[p_arr, col_arr] = gids
        slot_e1 = np.zeros((128, FW), np.int64)
        slot_e1[p_arr, col_arr] = e1g
        slot_u = np.full((128, FW), 0.5, np.float32)
        slot_u[p_arr, col_arr] = u[gids]
        # table idx of e1 (node-order A-region across cores)
        ip1 = NA * (slot_e1 // NLOC) + (slot_e1 % NLOC)
        ip1[slot_gid < 0] = 0

        # instance -> node map for the GEMM (local node ids; -1 = zero row)
        instA_node = np.arange(NA, dtype=np.int64)
        instA_node[NLOC:] = -1
        instB_node = -np.ones(NB, np.int64)
        instB_node[:nBc] = np.repeat(np.arange(NLOC), bcount)
        instC_node = -np.ones(NC_, np.int64)
        instC_node[:nCc] = np.repeat(np.arange(NLOC), ccount)
        inst_node = np.concatenate([
            instA_node, instB_node, instC_node,
            -np.ones(INSTP - INST, np.int64)])

        base = np.zeros((IN_DIM, INSTP), np.float32)
        valid = inst_node >= 0
        base[:, valid] = embedding[NLOC * c + inst_node[valid]].T
        embT = np.ascontiguousarray(
            base.reshape(4, 128, SLABS, 512).transpose(2, 0, 1, 3)
            .reshape(4 * SLABS * 128, 512)).astype(ml_dtypes.bfloat16)

        # gather stream: group g, j in [0, KG): slot (16g + j//FW, j%FW)
        # wrapped idx rw[16g + j%16, j//16]; chunk cu[p, j] = chunk of stream
        off = (ip1 % CHUNK).astype(np.int16)       # [128, FW]
        chk = (ip1 // CHUNK).astype(np.uint8)
        # stream per group: s_g[j] = off[16g + j//FW, j%FW]
        j = np.arange(KG)
        rw_full = np.empty((128, KG // 16), np.int16)
        cu_full = np.empty((128, KG), np.uint8)
        for g in range(8):
            sg_off = off[16 * g + j // FW, j % FW]
            sg_chk = chk[16 * g + j // FW, j % FW]
            rw_full[16 * g + j % 16, j // 16] = sg_off
            cu_full[16 * g:16 * (g + 1), :] = sg_chk[None, :]

        in_maps.append({
            "embT": embT,
            "Wt": W_bf,
            "bias2": bias2,
            "wsym2": wsym2_bf,
            "bedge": bedge,
            "iota16": iota16,
            "e8": e8,
            "rw": np.ascontiguousarray(rw_full),
            "cu": np.ascontiguousarray(cu_full),
            "uu": np.ascontiguousarray(slot_u),
        })
        slot_maps.append(slot_gid)
    return in_maps, slot_maps


def kernel(embedding, edges, u, W_emb, b_emb, W_edge, b_edge, _trace=False, _debug=False):
    key = (_debug,)
    if key not in _PROGRAM_CACHE:
        _PROGRAM_CACHE[key] = _build_program(_debug)
    nc = _PROGRAM_CACHE[key]
    in_maps, slot_maps = _prep_inputs(embedding, edges, u, W_emb, b_emb, W_edge, b_edge)
    res = run_bass_kernel_spmd(nc, in_maps, core_ids=list(range(NCORES)), trace=_trace)
    full = np.zeros(E, np.float32)
    for c in range(NCORES):
        o = res.results[c]["out"]
        sg = slot_maps[c]
        sel = sg >= 0
        full[sg[sel]] = o[sel]
    if _debug or _trace:
        kernel._last_results = res
        kernel._slot_maps = slot_maps
    return full
